# revision 28
# baseline (speedup 1.0000x reference)
"""Trainium2 Bass kernel for multi-head enc-dec attention with softmax over
the query axis (legacy F.softmax(dim=1) on [N, S, S]) plus an output
projection.

Math (per head n):
    S[i, j]  = sum_d Q[n, d, i] * K[n, d, j] / sqrt(128)
    E        = exp(S)                      (softmax over i == axis 0)
    U[d, j]  = sum_i V[n, d, i] * E[i, j]
    out_h    = U / colsum(E)               (colsum over i, per j)
    result[b] = sum_{heads h of b} W_h^T @ out_h

Sharding: N = 64 heads split across 8 cores (8 heads each). Each core
computes a partial projection over its 8 heads; cores 2b and 2b+1 hold the
two halves of batch b's heads, so the host just adds the two partial
projections per batch. No collectives.

Device-side layout per head (all fully unrolled, Tile-scheduled):
    mm1:  lhsT = Q[:, ic*128:+128] (fp32r), rhs = K-half (fp32r) -> S in PSUM
    exp:  ScalarE Exp with scale=1/sqrt(128), PSUM -> SBUF bf16 E chunk
    Esum: VectorE running sum of E chunks (bf16)
    mm2:  lhsT = V^T chunk (bf16), rhs = E chunk -> U accumulated in PSUM
    colsum: 8 matmuls ones[128,1]^T @ Esum-block -> [8, 128] PSUM
    recip:  VectorE reciprocal -> [8, 128] SBUF, cast bf16
    R:      8 outer-product matmuls ones[1,128]^T @ recip-row -> [128, 1024]
    scale:  VectorE U * R -> bf16 U_scaled
    proj:   lhsT = W_h (bf16), rhs = U_scaled -> accumulated over heads
"""

import math
from contextlib import ExitStack

import ml_dtypes
import numpy as np

N_CORES = 8
B, N_HEADS, D, S = 4, 16, 128, 2048
HPC = (B * N_HEADS) // N_CORES  # heads per core = 8
IC = S // 128  # 16 i-chunks
JH = 2  # j halves
JHALF = S // JH  # 1024
SCALE = 1.0 / math.sqrt(D)
QK_BF16 = True  # Q/K in bf16: enables FWL on mm1 weights (fp32 disables it)

_COMPILED = {}


def _build_nc(loop_n=None):
    """loop_n: if set, wrap the body in a device-side For_i that repeats it
    loop_n times (used only for wall-clock-difference HW timing)."""
    import contextlib
    import concourse.mybir as mybir
    import concourse.tile as tile
    from concourse import bacc

    from concourse.masks import make_identity

    F32 = mybir.dt.float32
    F32R = mybir.dt.float32r
    BF16 = mybir.dt.bfloat16
    EXP = mybir.ActivationFunctionType.Exp

    nc = bacc.Bacc("TRN2", target_bir_lowering=False, debug=False,
                   num_devices=N_CORES)

    qk_dt = BF16 if QK_BF16 else F32
    q_d = nc.dram_tensor("q", [HPC, D, S], qk_dt, kind="ExternalInput").ap()
    k_d = nc.dram_tensor("k", [HPC, D, S], qk_dt, kind="ExternalInput").ap()
    vt_d = nc.dram_tensor("vt", [HPC, D, S], BF16, kind="ExternalInput").ap()
    w_d = nc.dram_tensor("w", [D, HPC * D], BF16, kind="ExternalInput").ap()
    o_d = nc.dram_tensor("o", [D, S], F32, kind="ExternalOutput").ap()

    with tile.TileContext(nc) as tc:
        with ExitStack() as ctx:
            cpool = ctx.enter_context(tc.tile_pool(name="const", bufs=1))
            qpool = ctx.enter_context(tc.tile_pool(name="q", bufs=5))
            kpool = ctx.enter_context(tc.tile_pool(name="k", bufs=5))
            vtpool = ctx.enter_context(tc.tile_pool(name="vt", bufs=5))
            epool = ctx.enter_context(tc.tile_pool(name="e", bufs=8))
            esumpool = ctx.enter_context(tc.tile_pool(name="esum", bufs=2))
            rpool = ctx.enter_context(tc.tile_pool(name="recip", bufs=2))
            uspool = ctx.enter_context(tc.tile_pool(name="us", bufs=2))
            accpool = ctx.enter_context(tc.tile_pool(name="acc", bufs=2))
            spool = ctx.enter_context(
                tc.tile_pool(name="spsum", bufs=3, space="PSUM"))
            upool = ctx.enter_context(
                tc.tile_pool(name="upsum", bufs=1, space="PSUM"))

            first = _make_prefetch(nc, locals())(0, 0)

            ones_col = cpool.tile([128, 1], BF16, tag="ones_col")
            nc.vector.memset(ones_col[:], 1.0)
            ones_row = cpool.tile([1, 128], BF16, tag="ones_row")
            nc.vector.memset(ones_row[:], 1.0)
            ident = cpool.tile([128, 128], F32, tag="ident")
            make_identity(nc, ident[:])
            w_sb = cpool.tile([128, HPC * D], BF16, tag="w")
            nc.sync.dma_start(w_sb[:], w_d[:])

            loop_cm = (tc.For_i(0, loop_n, 1) if loop_n
                       else contextlib.nullcontext())
            with loop_cm:
                _emit_body(nc, tc, locals(), first=first)

    nc.compile()
    return nc


def _make_prefetch(nc, env):
    import concourse.mybir as mybir
    F32R = mybir.dt.float32r
    BF16 = mybir.dt.bfloat16
    qpool, kpool, vtpool = env["qpool"], env["kpool"], env["vtpool"]
    q_d, k_d, vt_d = env["q_d"], env["k_d"], env["vt_d"]

    QKDT = BF16 if QK_BF16 else F32R

    def prefetch(h, jh):
        # split loads across the SP and gpsimd DMA queues so early chunks'
        # inputs land before the whole transfer completes
        k = kpool.tile([128, JHALF], QKDT, tag="k")
        for p in range(2):
            nc.sync.dma_start(
                k[:, p * 512:(p + 1) * 512],
                k_d[h, :, jh * JHALF + p * 512:
                    jh * JHALF + (p + 1) * 512].bitcast(QKDT))
        q = qpool.tile([128, S], QKDT, tag="q")
        for p in range(4):
            eng = nc.sync if p % 2 == 0 else nc.gpsimd
            eng.dma_start(
                q[:, p * 512:(p + 1) * 512],
                q_d[h, :, p * 512:(p + 1) * 512].bitcast(QKDT))
        vt = vtpool.tile([128, S], BF16, tag="vt")
        for p in range(2):
            nc.gpsimd.dma_start(
                vt[:, p * 1024:(p + 1) * 1024],
                vt_d[h, :, p * 1024:(p + 1) * 1024])
        return q, k, vt

    return prefetch


def _emit_body(nc, tc, env, lag=8, first=None):
    """Fully software-pipelined emission.

    - mm2 (the V @ E accumulation) lags mm1/exp by `lag` chunks so the
      single U PSUM buffer frees (via the previous head's U-scale) before
      the next head's first mm2 reaches the PE queue.
    - Each head's normalization tail is split into 9 pieces woven into
      the NEXT head's chunk stream, each placed late enough that its
      cross-engine dependency is already satisfied when the in-order
      engine queue reaches it: no engine ever blocks.
    """
    import concourse.mybir as mybir

    F32 = mybir.dt.float32
    F32R = mybir.dt.float32r
    BF16 = mybir.dt.bfloat16
    EXP = mybir.ActivationFunctionType.Exp
    qpool, kpool, vtpool = env["qpool"], env["kpool"], env["vtpool"]
    epool, esumpool, rpool = env["epool"], env["esumpool"], env["rpool"]
    uspool, accpool, spool, upool = (
        env["uspool"], env["accpool"], env["spool"], env["upool"])
    ones_col, ones_row, ident, w_sb = (
        env["ones_col"], env["ones_row"], env["ident"], env["w_sb"])
    q_d, k_d, vt_d, o_d = env["q_d"], env["k_d"], env["vt_d"], env["o_d"]

    prefetch = _make_prefetch(nc, env)

    class Head:
        pass

    def tail_piece(st, piece):
        if piece == 0:
            # colsumT[js, jb] = sum_i Esum[i, jb*128+js]; colsumT and crow
            # share one borrowed S slot (bank-level deps serialize the
            # overlapping writes correctly)
            st.tail = spool.tile([128, JHALF], F32, tag="s")
            st.colsumT = st.tail[:, JHALF - 8:JHALF]
            st.crow = st.tail[0:1, :]
            for jb in range(8):
                nc.tensor.matmul(
                    st.colsumT[:, jb:jb + 1],
                    lhsT=st.esum[:, jb * 128:(jb + 1) * 128],
                    rhs=ones_col[:],
                    start=True, stop=True)
        elif piece == 1:
            st.recipT = rpool.tile([128, 8], F32, tag="recipT")
            nc.vector.reciprocal(st.recipT[:], st.colsumT[:])
        elif piece == 2:
            # transpose columns into [1, 128] row pieces (the last piece
            # overwrites the colsumT bytes — safe, recip has consumed them)
            for jb in range(8):
                nc.tensor.transpose(
                    st.crow[:, jb * 128:(jb + 1) * 128],
                    st.recipT[:, jb:jb + 1],
                    ident[:])
        elif piece == 3:
            st.recip_bf = rpool.tile([1, JHALF], BF16, tag="recipbf")
            nc.vector.tensor_copy(st.recip_bf[:], st.crow[:])
        elif piece == 4:
            # broadcast across partitions: R[p, j] = recip[j]
            st.R = spool.tile([128, JHALF], F32, tag="s")
            for js in range(2):
                nc.tensor.matmul(
                    st.R[:, js * 512:(js + 1) * 512],
                    lhsT=ones_row[:],
                    rhs=st.recip_bf[:, js * 512:(js + 1) * 512],
                    start=True, stop=True)
        elif piece == 5:
            # bounce u to SBUF: this is u's last reader, so the single U
            # PSUM buffer frees as soon as mm2(15) lands — independent of
            # the longer reciprocal chain
            st.u_sb = uspool.tile([128, JHALF], F32, tag="u_sb")
            nc.vector.tensor_copy(st.u_sb[:], st.u[:])
        elif piece == 6:
            st.us = uspool.tile([128, JHALF], BF16, tag="us")
            nc.vector.tensor_mul(st.us[:], st.u_sb[:], st.R[:])
        elif piece == 7:
            st.projp = spool.tile([128, JHALF], F32, tag="s")
            for js in range(2):
                nc.tensor.matmul(
                    st.projp[:, js * 512:(js + 1) * 512],
                    lhsT=w_sb[:, st.h * 128:(st.h + 1) * 128],
                    rhs=st.us[:, js * 512:(js + 1) * 512],
                    start=True, stop=True)
        elif piece == 8:
            if st.h == 0:
                nc.vector.tensor_copy(st.acc[:], st.projp[:])
            else:
                nc.vector.tensor_add(st.acc[:], st.acc[:], st.projp[:])
            if st.last_of_jh:
                nc.sync.dma_start(
                    o_d[:, st.jh * JHALF:(st.jh + 1) * JHALF], st.acc[:])

    PIECE_CHUNK = [1, 2, 3, 4, 5, 8, 9, 10, 11]
    steps = [(h, jh) for jh in range(JH) for h in range(HPC)]
    cur = first if first is not None else prefetch(*steps[0])
    nxt = None
    pend = None          # Head state awaiting its tail pieces
    tail_next = 0        # next tail piece to emit
    mm2q = []            # lagged (vt, e, u, ic) mm2 work
    acc = None

    def emit_mm2(ent):
        vt, e, u, ic = ent
        for js in range(2):
            nc.tensor.matmul(
                u[:, js * 512:(js + 1) * 512],
                lhsT=vt[:, ic * 128:(ic + 1) * 128],
                rhs=e[:, js * 512:(js + 1) * 512],
                start=(ic == 0), stop=(ic == IC - 1))

    for si, (h, jh) in enumerate(steps):
        if h == 0:
            acc = accpool.tile([128, JHALF], F32, tag="acc")
        q, k, vt = cur
        u = upool.tile([128, JHALF], F32, tag="u")
        esum = esumpool.tile([128, JHALF], BF16, tag="esum")
        esum_b = esumpool.tile([128, JHALF], BF16, tag="esum_b")

        for ic in range(IC):
            s = spool.tile([128, JHALF], F32, tag="s")
            for js in range(2):
                nc.tensor.matmul(
                    s[:, js * 512:(js + 1) * 512],
                    lhsT=q[:, ic * 128:(ic + 1) * 128],
                    rhs=k[:, js * 512:(js + 1) * 512],
                    start=True, stop=True)
            e = epool.tile([128, JHALF], BF16, tag="e")
            nc.scalar.activation(e[:], s[:], EXP, scale=SCALE)
            # split the running sum: 4 chunks accumulate on gpsimd to
            # offload the vector engine; merged below at chunk 15
            if ic == 0:
                nc.vector.tensor_copy(esum[:], e[:])
            elif ic == 12:
                nc.gpsimd.tensor_copy(esum_b[:], e[:])
            elif ic > 12:
                nc.gpsimd.tensor_add(esum_b[:], esum_b[:], e[:])
            else:
                nc.vector.tensor_add(esum[:], esum[:], e[:])
            if ic == IC - 1:
                nc.vector.tensor_add(esum[:], esum[:], esum_b[:])
            mm2q.append((vt, e, u, ic))
            if len(mm2q) > lag:
                emit_mm2(mm2q.pop(0))
            while (pend is not None and tail_next < 9
                   and ic >= PIECE_CHUNK[tail_next]):
                tail_piece(pend, tail_next)
                tail_next += 1
            if ic == 1 and si + 1 < len(steps):
                nxt = prefetch(*steps[si + 1])
        assert pend is None or tail_next >= 9
        st = Head()
        st.esum, st.u, st.acc, st.h, st.jh = esum, u, acc, h, jh
        st.last_of_jh = (h == HPC - 1)
        pend, tail_next = st, 0
        cur = nxt

    # drain: remaining lagged mm2s, then the last head's tail
    for ent in mm2q:
        emit_mm2(ent)
    for piece in range(9):
        tail_piece(pend, piece)


def _get_nc():
    if "nc" not in _COMPILED:
        _COMPILED["nc"] = _build_nc()
    return _COMPILED["nc"]


def _prep_inputs(Q, K, V, W):
    """Slice + lay out per-core inputs on host."""
    bf16 = ml_dtypes.bfloat16
    Q = np.ascontiguousarray(Q, dtype=np.float32)
    K = np.ascontiguousarray(K, dtype=np.float32)
    V = np.ascontiguousarray(V, dtype=np.float32)
    W = np.ascontiguousarray(W, dtype=np.float32)

    in_maps = []
    for c in range(N_CORES):
        qs = Q[c * HPC:(c + 1) * HPC]
        ks = K[c * HPC:(c + 1) * HPC]
        vs = V[c * HPC:(c + 1) * HPC]
        # vt[h][i_sub, ic*128 + d] = V[h, d, ic*128 + i_sub]
        vt = (vs.reshape(HPC, D, IC, 128)
              .transpose(0, 3, 2, 1)
              .reshape(HPC, 128, S)
              .astype(bf16))
        # w[d, h*128 + dd] = W[(c%2)*1024 + h*128 + d, dd]
        wc = W[(c % 2) * (HPC * D):((c % 2) + 1) * (HPC * D)]
        wt = (wc.reshape(HPC, D, D).transpose(1, 0, 2)
              .reshape(D, HPC * D).astype(bf16))
        if QK_BF16:
            qs = qs.astype(bf16)
            ks = ks.astype(bf16)
        in_maps.append({
            "q": np.ascontiguousarray(qs),
            "k": np.ascontiguousarray(ks),
            "vt": np.ascontiguousarray(vt),
            "w": np.ascontiguousarray(wt),
        })
    return in_maps


def _run(in_maps, trace=False):
    from concourse.bass_utils import run_bass_kernel_spmd
    nc = _get_nc()
    return run_bass_kernel_spmd(nc, in_maps, list(range(N_CORES)), trace=trace)


def kernel(x, Q, K, V, W, _trace=False, _return_result=False):
    in_maps = _prep_inputs(Q, K, V, W)
    res = _run(in_maps, trace=_trace)
    out = np.zeros((B, D, S), dtype=np.float32)
    for c in range(N_CORES):
        out[c // 2] += res.results[c]["o"]
    if _return_result:
        return out, res
    return out


# revision 30
# speedup vs baseline: 1.2850x; 1.2850x over previous
"""Trainium2 Bass kernel for multi-head enc-dec attention with softmax over
the query axis (legacy F.softmax(dim=1) on [N, S, S]) plus an output
projection.

Math (per head n):
    S[i, j]  = sum_d Q[n, d, i] * K[n, d, j] / sqrt(128)
    E        = exp(S)                      (softmax over i == axis 0)
    U[d, j]  = sum_i V[n, d, i] * E[i, j]
    out_h    = U / colsum(E)               (colsum over i, per j)
    result[b] = sum_{heads h of b} W_h^T @ out_h

Sharding: N = 64 heads split across 8 cores (8 heads each). Each core
computes a partial projection over its 8 heads; cores 2b and 2b+1 hold the
two halves of batch b's heads, so the host just adds the two partial
projections per batch. No collectives.

Device-side layout per head (all fully unrolled, Tile-scheduled):
    mm1:  lhsT = Q[:, ic*128:+128] (fp32r), rhs = K-half (fp32r) -> S in PSUM
    exp:  ScalarE Exp with scale=1/sqrt(128), PSUM -> SBUF bf16 E chunk
    Esum: VectorE running sum of E chunks (bf16)
    mm2:  lhsT = V^T chunk (bf16), rhs = E chunk -> U accumulated in PSUM
    colsum: 8 matmuls ones[128,1]^T @ Esum-block -> [8, 128] PSUM
    recip:  VectorE reciprocal -> [8, 128] SBUF, cast bf16
    R:      8 outer-product matmuls ones[1,128]^T @ recip-row -> [128, 1024]
    scale:  VectorE U * R -> bf16 U_scaled
    proj:   lhsT = W_h (bf16), rhs = U_scaled -> accumulated over heads
"""

import math
from contextlib import ExitStack

import ml_dtypes
import numpy as np

N_CORES = 8
B, N_HEADS, D, S = 4, 16, 128, 2048
HPC = (B * N_HEADS) // N_CORES  # heads per core = 8
IC = S // 128  # 16 i-chunks
JH = 2  # j halves
JHALF = S // JH  # 1024
SCALE = 1.0 / math.sqrt(D)
QK_BF16 = True  # Q/K in bf16: enables FWL on mm1 weights (fp32 disables it)

_COMPILED = {}


def _build_nc(loop_n=None):
    """loop_n: if set, wrap the body in a device-side For_i that repeats it
    loop_n times (used only for wall-clock-difference HW timing)."""
    import contextlib
    import concourse.mybir as mybir
    import concourse.tile as tile
    from concourse import bacc

    from concourse.masks import make_identity

    F32 = mybir.dt.float32
    F32R = mybir.dt.float32r
    BF16 = mybir.dt.bfloat16
    EXP = mybir.ActivationFunctionType.Exp

    nc = bacc.Bacc("TRN2", target_bir_lowering=False, debug=False,
                   num_devices=N_CORES)

    qk_dt = BF16 if QK_BF16 else F32
    q_d = nc.dram_tensor("q", [HPC, D, S], qk_dt, kind="ExternalInput").ap()
    k_d = nc.dram_tensor("k", [HPC, D, S], qk_dt, kind="ExternalInput").ap()
    vt_d = nc.dram_tensor("vt", [HPC, D, S], BF16, kind="ExternalInput").ap()
    w_d = nc.dram_tensor("w", [D, HPC * D], BF16, kind="ExternalInput").ap()
    o_d = nc.dram_tensor("o", [D, S], F32, kind="ExternalOutput").ap()

    with tile.TileContext(nc) as tc:
        with ExitStack() as ctx:
            cpool = ctx.enter_context(tc.tile_pool(name="const", bufs=1))
            qpool = ctx.enter_context(tc.tile_pool(name="q", bufs=5))
            kpool = ctx.enter_context(tc.tile_pool(name="k", bufs=5))
            vtpool = ctx.enter_context(tc.tile_pool(name="vt", bufs=5))
            epool = ctx.enter_context(tc.tile_pool(name="e", bufs=8))
            esumpool = ctx.enter_context(tc.tile_pool(name="esum", bufs=2))
            rpool = ctx.enter_context(tc.tile_pool(name="recip", bufs=2))
            uspool = ctx.enter_context(tc.tile_pool(name="us", bufs=2))
            accpool = ctx.enter_context(tc.tile_pool(name="acc", bufs=2))
            spool = ctx.enter_context(
                tc.tile_pool(name="spsum", bufs=3, space="PSUM"))
            upool = ctx.enter_context(
                tc.tile_pool(name="upsum", bufs=1, space="PSUM"))

            first = _make_prefetch(nc, locals())(0, 0)

            ones_col = cpool.tile([128, 1], BF16, tag="ones_col")
            nc.vector.memset(ones_col[:], 1.0)
            ones_row = cpool.tile([1, 128], BF16, tag="ones_row")
            nc.vector.memset(ones_row[:], 1.0)
            ident = cpool.tile([128, 128], F32, tag="ident")
            make_identity(nc, ident[:])
            w_sb = cpool.tile([128, HPC * D], BF16, tag="w")
            nc.sync.dma_start(w_sb[:], w_d[:])

            loop_cm = (tc.For_i(0, loop_n, 1) if loop_n
                       else contextlib.nullcontext())
            with loop_cm:
                _emit_body(nc, tc, locals(), first=first)

    nc.compile()
    return nc


def _make_prefetch(nc, env):
    import concourse.mybir as mybir
    F32R = mybir.dt.float32r
    BF16 = mybir.dt.bfloat16
    qpool, kpool, vtpool = env["qpool"], env["kpool"], env["vtpool"]
    q_d, k_d, vt_d = env["q_d"], env["k_d"], env["vt_d"]

    QKDT = BF16 if QK_BF16 else F32R

    def prefetch(h, jh):
        # split loads across the SP and gpsimd DMA queues so early chunks'
        # inputs land before the whole transfer completes
        k = kpool.tile([128, JHALF], QKDT, tag="k")
        for p in range(2):
            nc.sync.dma_start(
                k[:, p * 512:(p + 1) * 512],
                k_d[h, :, jh * JHALF + p * 512:
                    jh * JHALF + (p + 1) * 512].bitcast(QKDT))
        q = qpool.tile([128, S], QKDT, tag="q")
        for p in range(4):
            nc.sync.dma_start(
                q[:, p * 512:(p + 1) * 512],
                q_d[h, :, p * 512:(p + 1) * 512].bitcast(QKDT))
        vt = vtpool.tile([128, S], BF16, tag="vt")
        for p in range(2):
            nc.sync.dma_start(
                vt[:, p * 1024:(p + 1) * 1024],
                vt_d[h, :, p * 1024:(p + 1) * 1024])
        return q, k, vt

    return prefetch


def _emit_body(nc, tc, env, lag=8, first=None):
    """Fully software-pipelined emission.

    - mm2 (the V @ E accumulation) lags mm1/exp by `lag` chunks so the
      single U PSUM buffer frees (via the previous head's U-scale) before
      the next head's first mm2 reaches the PE queue.
    - Each head's normalization tail is split into 9 pieces woven into
      the NEXT head's chunk stream, each placed late enough that its
      cross-engine dependency is already satisfied when the in-order
      engine queue reaches it: no engine ever blocks.
    """
    import concourse.mybir as mybir

    F32 = mybir.dt.float32
    F32R = mybir.dt.float32r
    BF16 = mybir.dt.bfloat16
    EXP = mybir.ActivationFunctionType.Exp
    qpool, kpool, vtpool = env["qpool"], env["kpool"], env["vtpool"]
    epool, esumpool, rpool = env["epool"], env["esumpool"], env["rpool"]
    uspool, accpool, spool, upool = (
        env["uspool"], env["accpool"], env["spool"], env["upool"])
    ones_col, ones_row, ident, w_sb = (
        env["ones_col"], env["ones_row"], env["ident"], env["w_sb"])
    q_d, k_d, vt_d, o_d = env["q_d"], env["k_d"], env["vt_d"], env["o_d"]

    prefetch = _make_prefetch(nc, env)

    class Head:
        pass

    def tail_piece(st, piece):
        if piece == 0:
            # colsumT[js, jb] = sum_i Esum[i, jb*128+js]; colsumT and crow
            # share one borrowed S slot (bank-level deps serialize the
            # overlapping writes correctly)
            st.tail = spool.tile([128, JHALF], F32, tag="s")
            st.colsumT = st.tail[:, JHALF - 8:JHALF]
            st.crow = st.tail[0:1, :]
            for jb in range(8):
                nc.tensor.matmul(
                    st.colsumT[:, jb:jb + 1],
                    lhsT=st.esum[:, jb * 128:(jb + 1) * 128],
                    rhs=ones_col[:],
                    start=True, stop=True)
        elif piece == 1:
            st.recipT = rpool.tile([128, 8], F32, tag="recipT")
            nc.vector.reciprocal(st.recipT[:], st.colsumT[:])
        elif piece == 2:
            # transpose columns into [1, 128] row pieces (the last piece
            # overwrites the colsumT bytes — safe, recip has consumed them)
            for jb in range(8):
                nc.tensor.transpose(
                    st.crow[:, jb * 128:(jb + 1) * 128],
                    st.recipT[:, jb:jb + 1],
                    ident[:])
        elif piece == 3:
            st.recip_bf = rpool.tile([1, JHALF], BF16, tag="recipbf")
            nc.vector.tensor_copy(st.recip_bf[:], st.crow[:])
        elif piece == 4:
            # broadcast across partitions: R[p, j] = recip[j]
            st.R = spool.tile([128, JHALF], F32, tag="s")
            for js in range(2):
                nc.tensor.matmul(
                    st.R[:, js * 512:(js + 1) * 512],
                    lhsT=ones_row[:],
                    rhs=st.recip_bf[:, js * 512:(js + 1) * 512],
                    start=True, stop=True)
        elif piece == 5:
            # bounce u to SBUF: this is u's last reader, so the single U
            # PSUM buffer frees as soon as mm2(15) lands — independent of
            # the longer reciprocal chain
            st.u_sb = uspool.tile([128, JHALF], F32, tag="u_sb")
            nc.vector.tensor_copy(st.u_sb[:], st.u[:])
        elif piece == 6:
            st.us = uspool.tile([128, JHALF], BF16, tag="us")
            nc.vector.tensor_mul(st.us[:], st.u_sb[:], st.R[:])
        elif piece == 7:
            st.projp = spool.tile([128, JHALF], F32, tag="s")
            for js in range(2):
                nc.tensor.matmul(
                    st.projp[:, js * 512:(js + 1) * 512],
                    lhsT=w_sb[:, st.h * 128:(st.h + 1) * 128],
                    rhs=st.us[:, js * 512:(js + 1) * 512],
                    start=True, stop=True)
        elif piece == 8:
            if st.h == 0:
                nc.vector.tensor_copy(st.acc[:], st.projp[:])
            else:
                nc.vector.tensor_add(st.acc[:], st.acc[:], st.projp[:])
            if st.last_of_jh:
                nc.sync.dma_start(
                    o_d[:, st.jh * JHALF:(st.jh + 1) * JHALF], st.acc[:])

    PIECE_CHUNK = [1, 2, 3, 4, 5, 8, 9, 10, 11]
    steps = [(h, jh) for jh in range(JH) for h in range(HPC)]
    cur = first if first is not None else prefetch(*steps[0])
    nxt = None
    pend = None          # Head state awaiting its tail pieces
    tail_next = 0        # next tail piece to emit
    mm2q = []            # lagged (vt, e, u, ic) mm2 work
    acc = None

    def emit_mm2(ent):
        vt, e, u, ic = ent
        for js in range(2):
            nc.tensor.matmul(
                u[:, js * 512:(js + 1) * 512],
                lhsT=vt[:, ic * 128:(ic + 1) * 128],
                rhs=e[:, js * 512:(js + 1) * 512],
                start=(ic == 0), stop=(ic == IC - 1))

    for si, (h, jh) in enumerate(steps):
        if h == 0:
            acc = accpool.tile([128, JHALF], F32, tag="acc")
        q, k, vt = cur
        u = upool.tile([128, JHALF], F32, tag="u")
        esum = esumpool.tile([128, JHALF], BF16, tag="esum")

        for ic in range(IC):
            s = spool.tile([128, JHALF], F32, tag="s")
            for js in range(2):
                nc.tensor.matmul(
                    s[:, js * 512:(js + 1) * 512],
                    lhsT=q[:, ic * 128:(ic + 1) * 128],
                    rhs=k[:, js * 512:(js + 1) * 512],
                    start=True, stop=True)
            e = epool.tile([128, JHALF], BF16, tag="e")
            nc.scalar.activation(e[:], s[:], EXP, scale=SCALE)
            if ic == 0:
                nc.vector.tensor_copy(esum[:], e[:])
            else:
                nc.vector.tensor_add(esum[:], esum[:], e[:])
            mm2q.append((vt, e, u, ic))
            if len(mm2q) > lag:
                emit_mm2(mm2q.pop(0))
            while (pend is not None and tail_next < 9
                   and ic >= PIECE_CHUNK[tail_next]):
                tail_piece(pend, tail_next)
                tail_next += 1
            if ic == 1 and si + 1 < len(steps):
                nxt = prefetch(*steps[si + 1])
        assert pend is None or tail_next >= 9
        st = Head()
        st.esum, st.u, st.acc, st.h, st.jh = esum, u, acc, h, jh
        st.last_of_jh = (h == HPC - 1)
        pend, tail_next = st, 0
        cur = nxt

    # drain: remaining lagged mm2s, then the last head's tail
    for ent in mm2q:
        emit_mm2(ent)
    for piece in range(9):
        tail_piece(pend, piece)


def _get_nc():
    if "nc" not in _COMPILED:
        _COMPILED["nc"] = _build_nc()
    return _COMPILED["nc"]


def _prep_inputs(Q, K, V, W):
    """Slice + lay out per-core inputs on host."""
    bf16 = ml_dtypes.bfloat16
    Q = np.ascontiguousarray(Q, dtype=np.float32)
    K = np.ascontiguousarray(K, dtype=np.float32)
    V = np.ascontiguousarray(V, dtype=np.float32)
    W = np.ascontiguousarray(W, dtype=np.float32)

    in_maps = []
    for c in range(N_CORES):
        qs = Q[c * HPC:(c + 1) * HPC]
        ks = K[c * HPC:(c + 1) * HPC]
        vs = V[c * HPC:(c + 1) * HPC]
        # vt[h][i_sub, ic*128 + d] = V[h, d, ic*128 + i_sub]
        vt = (vs.reshape(HPC, D, IC, 128)
              .transpose(0, 3, 2, 1)
              .reshape(HPC, 128, S)
              .astype(bf16))
        # w[d, h*128 + dd] = W[(c%2)*1024 + h*128 + d, dd]
        wc = W[(c % 2) * (HPC * D):((c % 2) + 1) * (HPC * D)]
        wt = (wc.reshape(HPC, D, D).transpose(1, 0, 2)
              .reshape(D, HPC * D).astype(bf16))
        if QK_BF16:
            qs = qs.astype(bf16)
            ks = ks.astype(bf16)
        in_maps.append({
            "q": np.ascontiguousarray(qs),
            "k": np.ascontiguousarray(ks),
            "vt": np.ascontiguousarray(vt),
            "w": np.ascontiguousarray(wt),
        })
    return in_maps


def _run(in_maps, trace=False):
    from concourse.bass_utils import run_bass_kernel_spmd
    nc = _get_nc()
    return run_bass_kernel_spmd(nc, in_maps, list(range(N_CORES)), trace=trace)


def kernel(x, Q, K, V, W, _trace=False, _return_result=False):
    in_maps = _prep_inputs(Q, K, V, W)
    res = _run(in_maps, trace=_trace)
    out = np.zeros((B, D, S), dtype=np.float32)
    for c in range(N_CORES):
        out[c // 2] += res.results[c]["o"]
    if _return_result:
        return out, res
    return out
